# revision 2
# baseline (speedup 1.0000x reference)
"""3x3 valid cross-correlation of a 4096x4096 fp32 image + scalar bias,
sharded row-wise across 8 TRN2 NeuronCores.

bf16 datapath (harness gate is rel_err < 2e-2; bf16 lands ~5e-3):
  - x is cast to bf16 on host -> load DMA traffic halves (4.2 MB/core).
  - Matmuls run bf16 x bf16 -> fp32 PSUM at 1 cycle/column.
  - Output is stored as bf16 (4.2 MB/core) and upcast to fp32 on host.

Strategy per core (512 output rows, 514 input rows incl. 2-row halo taken
host-side via overlapping slices -- no device collectives):
  - Row panels of 128 input rows -> 126 output rows (banded matmul):
    out[m, n] = sum_dc sum_dr w[dr, dc] * x[m+dr, n+dc]
    For each kernel column dc, a banded stationary matrix
    B_dc[k, m] = w[k-m, dc] (k-m in 0..2) gives
    psum[m, n] += sum_k B_dc[k, m] * x[k, n+dc].
  - The 8-row tail (rows 504..512) is packed 12 column-blocks deep into
    the PE contraction dim (stationary [120, 96] block-diagonal banded,
    moving [120, 345] gathered on host); its 3 matmuls run right after
    panel 0 so nothing but panel 3's last chunk sits at the kernel end.
  - Panel 0 is loaded in four 1026-column pieces so chunk 0's matmuls
    can start as soon as ~260 KB has landed (vs ~1 MB for the full
    panel).  This shrinks the DVFS-warmup bridge (zero-matmuls that keep
    the PE busy from its first possible cycle until real data arrives)
    from 15 matmuls to 7.
  - Scheduling is dominated by the NC activity manager (HAM): DMA is
    capped at ~230-270 GB/s and the PE at ~1.2 GHz until ~5-6us of
    sustained activity earn the full-rate grant (~430 GB/s, 2.4 GHz),
    and any PE idle gap early in the run triggers a half-rate (k=4/8)
    throttle spiral.  The warmup matmuls therefore bridge the PE from
    its first possible cycle (~8us, after the fixed SPMD prologue).
  - PSUM is drained per 1024-col chunk (VectorE on even chunks, ScalarE
    on odd -- ScalarE's activation drain is ~2.4x faster, so it gets the
    chunk on the critical path), with the bias fused in, converting to
    bf16.
  - Stores go out per chunk ([126, 1024] bf16 = 2 KB packets) instead of
    per panel, so the end-of-kernel drain is one 258 KB chunk rather
    than a 1 MB panel.  Queueing keeps stores from starving the panel
    loads: panels 0/1 + tail store on the sync queue (FIFO behind the
    loads), panels 2/3 on the gpsimd queue (issued only after loads are
    done); panel 3's chunks are split across both queues to halve the
    final drain.
  - Last core overlaps core 6 by 2 rows so all cores run an identical
    514-row program (4094 = 8*512 - 2).
"""

import numpy as np
import ml_dtypes

import concourse.bacc as bacc
import concourse.mybir as mybir
from concourse import tile
from concourse.bass_utils import run_bass_kernel_spmd

H, W = 4096, 4096
KH, KW = 3, 3
OH, OW = H - KH + 1, W - KW + 1  # 4094, 4094
NCORES = 8
ROWS_PER_CORE = 512              # output rows computed per core
IN_ROWS = ROWS_PER_CORE + KH - 1  # 514 input rows per core
PANEL_OUT = 126                  # output rows per full 128-input-row panel
N_FULL_PANELS = 4                # 4 * 126 = 504
TAIL_OUT = ROWS_PER_CORE - N_FULL_PANELS * PANEL_OUT  # 8
TAIL_IN = TAIL_OUT + KH - 1      # 10
COLS_PER_MM = 512                # PSUM-bank max (512 fp32)
CHUNK = 1024                     # PSUM chunk = 2 banks
# Packed tail geometry: 12 column blocks, stride 341, 343 output columns
# each; 341*11 + 343 = 4094 exactly, and input reads stop at 4096.
TJ = 12
TSTRIDE = 341
TN = 343
WARMUP_MM = 7
# Panel-0 column pieces: chunk c's matmuls read cols [1024c, 1024c+1026);
# piece boundaries at 1026/2050/3074 keep every chunk's read window inside
# the union of pieces loaded so far.
P0_PIECES = [(0, 1026), (1026, 2050), (2050, 3074), (3074, 4096)]

_F32 = mybir.dt.float32
_BF16 = mybir.dt.bfloat16
BF = ml_dtypes.bfloat16

_PROGRAM_CACHE = None
last_results = None  # BassKernelResults of the most recent kernel() call


def _build_program():
    nc = bacc.Bacc(
        "TRN2", target_bir_lowering=False, debug=False, num_devices=NCORES
    )
    x = nc.dram_tensor("x", [IN_ROWS, W], _BF16, kind="ExternalInput")
    xt_p = nc.dram_tensor("xt", [TJ * TAIL_IN, TN + KW - 1], _BF16,
                          kind="ExternalInput")
    w = nc.dram_tensor("w", [128, KW * PANEL_OUT], _BF16, kind="ExternalInput")
    wt_p = nc.dram_tensor("wt", [TJ * TAIL_IN, KW * TJ * TAIL_OUT], _BF16,
                          kind="ExternalInput")
    b = nc.dram_tensor("b", [128, 1], _F32, kind="ExternalInput")
    # y cols are padded to 4096 so chunk stores are uniform 1024-col (2 KB
    # packet) writes.  Host slices off the 2 pad columns.
    y = nc.dram_tensor("y", [N_FULL_PANELS * PANEL_OUT, W], _BF16,
                       kind="ExternalOutput")
    yt = nc.dram_tensor("yt", [TJ * TAIL_OUT, TN], _BF16,
                        kind="ExternalOutput")

    TK = TJ * TAIL_IN   # 120
    TM = TJ * TAIL_OUT  # 96

    with tile.TileContext(nc) as tc:
        with (
            tc.tile_pool(name="const", bufs=1) as cpool,
            tc.tile_pool(name="xp", bufs=4) as xpool,
            tc.tile_pool(name="op", bufs=10) as opool,
            tc.tile_pool(name="pp", bufs=4, space="PSUM") as ppool,
        ):
            # Warmup memset first on gpsimd (its queue only carries late
            # stores), so the PE can start at once.
            wz = cpool.tile([128, COLS_PER_MM], _BF16)
            nc.gpsimd.memset(wz[:], 0.0)

            # All loads ride the sync queue.  Order = need order: weights,
            # then panel-0 pieces, then bias, panel 1, tail operands,
            # panels 2-3.  Pieces are 128 x 2052 B; full panels are
            # full-width (8 KB packets, the max-rate shape).
            xts = []
            for panel in range(N_FULL_PANELS):
                xt = xpool.tile([128, W], _BF16)
                xts.append(xt)
            wt = cpool.tile([128, KW * PANEL_OUT], _BF16)
            nc.sync.dma_start(wt[:], w[:])
            for (a, bnd) in P0_PIECES:
                nc.sync.dma_start(xts[0][:, a:bnd], x[0:128, a:bnd])
            bt = cpool.tile([128, 1], _F32)
            nc.sync.dma_start(bt[:], b[:])
            r1 = PANEL_OUT
            nc.sync.dma_start(xts[1][:], x[r1 : r1 + 128, :])
            wtt = cpool.tile([TK, KW * TM], _BF16)
            nc.sync.dma_start(wtt[:], wt_p[:])
            xtt = cpool.tile([TK, TN + KW - 1], _BF16)
            nc.sync.dma_start(xtt[:], xt_p[:])
            for panel in range(2, N_FULL_PANELS):
                r0 = PANEL_OUT * panel
                nc.sync.dma_start(xts[panel][:], x[r0 : r0 + 128, :])

            # PE warmup on zeroed tiles: keeps the PE busy (DVFS ramping)
            # while panel 0's first piece streams in.
            psw = ppool.tile([128, CHUNK], _F32, tag="ps")
            for _ in range(WARMUP_MM):
                nc.tensor.matmul(
                    psw[:126, :COLS_PER_MM],
                    wz[:, :126],
                    wz[:, :],
                    start=True,
                    stop=True,
                )

            def do_panel(panel):
                r0 = PANEL_OUT * panel
                xt = xts[panel]
                for c in range(4):
                    ps = ppool.tile([128, CHUNK], _F32, tag="ps")
                    s0 = c * CHUNK
                    sw = min(CHUNK, OW - s0)  # 1024 / 1022
                    for dc in range(KW):
                        for jj in range(2):
                            c0 = s0 + jj * COLS_PER_MM
                            N = min(COLS_PER_MM, OW - c0)
                            lc0 = jj * COLS_PER_MM
                            nc.tensor.matmul(
                                ps[:PANEL_OUT, lc0 : lc0 + N],
                                wt[:128, dc * PANEL_OUT : dc * PANEL_OUT + PANEL_OUT],
                                xt[:128, c0 + dc : c0 + dc + N],
                                start=(dc == 0),
                                stop=(dc == KW - 1),
                            )
                    ot = opool.tile([128, CHUNK], _BF16)
                    if sw < CHUNK:
                        # Pad cols 1022:1024 of the last chunk so the full
                        # 2 KB store row is initialized.
                        nc.vector.memset(ot[:PANEL_OUT, sw:CHUNK], 0.0)
                    # Drain PSUM: ScalarE (fast activation) on odd chunks
                    # incl. the critical last one, VectorE on even chunks.
                    if c % 2 == 1:
                        nc.scalar.activation(
                            ot[:PANEL_OUT, :sw],
                            ps[:PANEL_OUT, :sw],
                            mybir.ActivationFunctionType.Identity,
                            bias=bt[:PANEL_OUT, :],
                        )
                    else:
                        nc.vector.tensor_scalar_add(
                            ot[:PANEL_OUT, :sw],
                            ps[:PANEL_OUT, :sw],
                            bt[:PANEL_OUT, :],
                        )
                    # Per-chunk store.  Panels 0/1 ride the sync queue
                    # (FIFO behind the remaining loads, so they never
                    # starve them); panel 2 rides gpsimd (loads done by
                    # then); panel 3 splits each chunk across both queues
                    # so the final drain is halved.
                    if panel <= 1:
                        nc.sync.dma_start(
                            y[r0 : r0 + PANEL_OUT, s0 : s0 + CHUNK],
                            ot[:PANEL_OUT, :CHUNK],
                        )
                    elif panel == 2:
                        nc.gpsimd.dma_start(
                            y[r0 : r0 + PANEL_OUT, s0 : s0 + CHUNK],
                            ot[:PANEL_OUT, :CHUNK],
                        )
                    else:
                        nc.sync.dma_start(
                            y[r0 : r0 + PANEL_OUT, s0 : s0 + COLS_PER_MM],
                            ot[:PANEL_OUT, :COLS_PER_MM],
                        )
                        nc.gpsimd.dma_start(
                            y[r0 : r0 + PANEL_OUT, s0 + COLS_PER_MM : s0 + CHUNK],
                            ot[:PANEL_OUT, COLS_PER_MM:CHUNK],
                        )

            do_panel(0)
            # Packed tail right after panel 0: one 3-matmul group covers
            # all 8 tail rows; its store rides the sync queue behind the
            # loads.
            pst = ppool.tile([128, CHUNK], _F32, tag="ps")
            for dc in range(KW):
                nc.tensor.matmul(
                    pst[:TM, :TN],
                    wtt[:TK, dc * TM : dc * TM + TM],
                    xtt[:TK, dc : dc + TN],
                    start=(dc == 0),
                    stop=(dc == KW - 1),
                )
            ott = opool.tile([TM, TN], _BF16)
            nc.scalar.activation(
                ott[:TM, :TN],
                pst[:TM, :TN],
                mybir.ActivationFunctionType.Identity,
                bias=bt[:TM, :],
            )
            nc.sync.dma_start(yt[:, :], ott[:TM, :TN])
            for panel in range(1, N_FULL_PANELS):
                do_panel(panel)

    nc.compile()
    return nc


def _banded_weights(weight: np.ndarray) -> np.ndarray:
    """lhsT for each kernel column dc, laid out as [128, KW*PANEL_OUT].

    wT[k, dc*PANEL_OUT + m] = weight[k - m, dc] for 0 <= k - m < KH.
    """
    wT = np.zeros((128, KW * PANEL_OUT), np.float32)
    m = np.arange(PANEL_OUT)
    for dc in range(KW):
        for d in range(KH):
            wT[m + d, dc * PANEL_OUT + m] = weight[d, dc]
    return wT.astype(BF)


def _tail_weights(weight: np.ndarray) -> np.ndarray:
    """Block-diagonal banded stationary for the packed tail.

    S[10j + m + d, dc*96 + 8j + m] = weight[d, dc].
    """
    TK = TJ * TAIL_IN
    TM = TJ * TAIL_OUT
    S = np.zeros((TK, KW * TM), np.float32)
    m = np.arange(TAIL_OUT)
    for dc in range(KW):
        for j in range(TJ):
            for d in range(KH):
                S[TAIL_IN * j + m + d, dc * TM + TAIL_OUT * j + m] = weight[d, dc]
    return S.astype(BF)


def _install_ntff_hook():
    """Shim antenv.axon_hooks so run_bass_kernel_spmd(trace=True) can find
    the axon NTFF profiling hook (the image's antenv lacks axon_hooks)."""
    import sys
    import types

    try:
        from antenv.axon_hooks import get_axon_ntff_profile_hook  # noqa: F401

        return
    except ImportError:
        pass
    import antenv
    from trn_agent_boot.trn_boot import _ntff_profile_via_ctypes

    hook = _ntff_profile_via_ctypes("/opt/axon/libaxon_pjrt.so")
    mod = types.ModuleType("antenv.axon_hooks")
    mod._hook = hook
    mod.set_axon_ntff_profile_hook = lambda h: setattr(mod, "_hook", h)
    mod.get_axon_ntff_profile_hook = lambda: mod._hook
    sys.modules["antenv.axon_hooks"] = mod
    antenv.axon_hooks = mod


def kernel(x, weight, bias, _trace=False, _trace_cores=None):
    global _PROGRAM_CACHE, last_results
    if _trace:
        _install_ntff_hook()
    x = np.asarray(x, dtype=np.float32)
    weight = np.asarray(weight, dtype=np.float32)
    bias = np.asarray(bias, dtype=np.float32)

    if _PROGRAM_CACHE is None:
        _PROGRAM_CACHE = _build_program()
    nc = _PROGRAM_CACHE

    xbf = x.astype(BF)
    wT = _banded_weights(weight)
    wtail = _tail_weights(weight)
    bb = np.full((128, 1), bias[0], np.float32)

    in_maps = []
    for i in range(NCORES):
        r0 = i * ROWS_PER_CORE if i < NCORES - 1 else H - IN_ROWS
        xc = xbf[r0 : r0 + IN_ROWS]
        # Packed tail moving operand: partition 10j+i = tail input row i,
        # column block j (stride TSTRIDE, width TN+2).
        tr = xc[N_FULL_PANELS * PANEL_OUT :]  # rows 504..514
        xtp = np.stack(
            [tr[:, TSTRIDE * j : TSTRIDE * j + TN + KW - 1] for j in range(TJ)]
        ).reshape(TJ * TAIL_IN, TN + KW - 1)
        in_maps.append(
            {
                "x": np.ascontiguousarray(xc),
                "xt": np.ascontiguousarray(xtp),
                "w": wT,
                "wt": wtail,
                "b": bb,
            }
        )

    kwargs = {}
    if _trace:
        kwargs["trace"] = True
        kwargs["trace_cores"] = (
            list(range(NCORES)) if _trace_cores is None else _trace_cores
        )
    res = run_bass_kernel_spmd(nc, in_maps, core_ids=list(range(NCORES)), **kwargs)
    last_results = res

    out = np.empty((OH, OW), np.float32)
    for i in range(NCORES):
        r0 = i * ROWS_PER_CORE if i < NCORES - 1 else H - IN_ROWS
        yi = res.results[i]["y"][:, :OW].astype(np.float32)
        out[r0 : r0 + N_FULL_PANELS * PANEL_OUT] = yi
        # Unpack the packed tail: partition 8j+m = tail row m, col block j.
        yti = res.results[i]["yt"].astype(np.float32)
        for j in range(TJ):
            out[
                r0 + N_FULL_PANELS * PANEL_OUT : r0 + ROWS_PER_CORE,
                TSTRIDE * j : TSTRIDE * j + TN,
            ] = yti[TAIL_OUT * j : TAIL_OUT * (j + 1)]
    return out
